# revision 10
# baseline (speedup 1.0000x reference)
"""Trainium2 Bass kernel for nn_FLinear2d (per-channel double linear).

Math (see reference):
  u[b,i,o] = sum_s U3[o,i,s] * x[b,i,s] + bU[o]        (64 per-channel matmuls)
  z[b,o,t] = sum_i V3[t,o,i] * u[b,i,o] + bV[t]        (128 per-o matmuls)

Two SPMD launches over 8 cores:
  Stage A: shard C_in (8 channels/core).  Per (i, s-chunk):
      psum[o=128, b=64] += Uh[i,:,c,:].T @ Xh[i,:,c,:]   (fp32, K=128)
    accumulated over 32 s-chunks -> u_base[o, i, b].
  Stage B: shard C_out (16 o/core).  Biases folded into 2 extra contraction
    rows (row 64: ones -> bV via moving side; row 65: bU[o] -> sum_i V3).
    Per (o, t-tile): psum[t=128, b=64] = Vh[o][:, tt*128:+128].T @ S[o]
    (single matmul, K=66).

All DMAs are contiguous thanks to host-side layout transforms.
"""

import numpy as np
from contextlib import ExitStack

import concourse.bass as bass
import concourse.tile as tile
from concourse import bacc, mybir
from concourse.bass_utils import run_bass_kernel_spmd

F32 = mybir.dt.float32
N_CORES = 8
CORE_IDS = list(range(N_CORES))

B, CI, CO = 64, 64, 128
S_IN, S_OUT = 4096, 1024
NCH = 32            # s-chunks of 128
I_PER_CORE = CI // N_CORES     # 8
O_PER_CORE = CO // N_CORES     # 16
KB = 66             # contraction for stage B: 64 i + ones row + bU row
TT = S_OUT // 128   # 8 t-tiles per o

_cache = {}


def _build_stage_a(repeat=1):
    nc = bacc.Bacc("TRN2", target_bir_lowering=False, debug=False,
                   num_devices=N_CORES)
    uh = nc.dram_tensor("uh", [I_PER_CORE, 128, NCH, CO], F32,
                        kind="ExternalInput").ap()
    xh = nc.dram_tensor("xh", [I_PER_CORE, 128, NCH, B], F32,
                        kind="ExternalInput").ap()
    u_out = nc.dram_tensor("u_out", [CO, I_PER_CORE, B], F32,
                           kind="ExternalOutput").ap()

    with tile.TileContext(nc) as tc, ExitStack() as ctx:
        up = ctx.enter_context(tc.tile_pool(name="ut", bufs=3))
        xp = ctx.enter_context(tc.tile_pool(name="xt", bufs=3))
        pp = ctx.enter_context(
            tc.tile_pool(name="ps", bufs=2, space=bass.MemorySpace.PSUM))
        sp = ctx.enter_context(tc.tile_pool(name="usb", bufs=1))

        Q = NCH // 4
        for _ in range(repeat):
            u_sb = sp.tile([CO, I_PER_CORE, B], F32)
            for i in range(I_PER_CORE):
                ut = up.tile([128, NCH, CO], F32)
                for q in range(4):
                    # alternate U quarters across the two HWDGE rings
                    eng = nc.sync if q % 2 == 0 else nc.scalar
                    eng.dma_start(ut[:, q * Q:(q + 1) * Q, :],
                                  uh[i, :, q * Q:(q + 1) * Q, :])
                xt = xp.tile([128, NCH, B], F32)
                nc.gpsimd.dma_start(xt[:], xh[i])
                ps = pp.tile([CO, B], F32)
                for c in range(NCH):
                    nc.tensor.matmul(ps[:], ut[:, c, :], xt[:, c, :],
                                     start=(c == 0), stop=(c == NCH - 1))
                nc.vector.tensor_copy(u_sb[:, i, :], ps[:])
            nc.gpsimd.dma_start(u_out[:], u_sb[:])
    nc.compile()
    return nc


def _build_stage_b(repeat=1):
    nc = bacc.Bacc("TRN2", target_bir_lowering=False, debug=False,
                   num_devices=N_CORES)
    vh = nc.dram_tensor("vh", [O_PER_CORE, KB, S_OUT], F32,
                        kind="ExternalInput").ap()
    us = nc.dram_tensor("us", [O_PER_CORE, KB, B], F32,
                        kind="ExternalInput").ap()
    z_out = nc.dram_tensor("z_out", [O_PER_CORE, 128, TT, B], F32,
                           kind="ExternalOutput").ap()

    with tile.TileContext(nc) as tc, ExitStack() as ctx:
        vp = ctx.enter_context(tc.tile_pool(name="vt", bufs=6))
        usp = ctx.enter_context(tc.tile_pool(name="ust", bufs=1))
        pp = ctx.enter_context(
            tc.tile_pool(name="ps", bufs=4, space=bass.MemorySpace.PSUM))
        zp = ctx.enter_context(tc.tile_pool(name="zsb", bufs=6))

        for _ in range(repeat):
            us_all = usp.tile([KB, O_PER_CORE, B], F32)
            nc.gpsimd.dma_start(us_all[:], us.rearrange("j k b -> k j b"))
            for j in range(O_PER_CORE):
                vt = vp.tile([KB, S_OUT], F32)
                nc.sync.dma_start(vt[:], vh[j])
                ps = pp.tile([128, TT, B], F32)
                for tt in range(TT):
                    nc.tensor.matmul(ps[:, tt, :], vt[:, bass.ts(tt, 128)],
                                     us_all[:, j, :], start=True, stop=True)
                z_sb = zp.tile([128, TT, B], F32)
                nc.vector.tensor_copy(z_sb[:], ps[:])
                nc.scalar.dma_start(z_out[j], z_sb[:])
    nc.compile()
    return nc


def _get(name):
    if name not in _cache:
        _cache[name] = _build_stage_a() if name == "a" else _build_stage_b()
    return _cache[name]


def _run(nc, in_maps, attempts=3):
    last = None
    for k in range(attempts):
        try:
            return run_bass_kernel_spmd(nc, in_maps, CORE_IDS).results
        except Exception as e:     # transient axon/runtime hiccups
            last = e
            if k + 1 < attempts:
                import time as _t
                _t.sleep(15 * (k + 1))
    raise last


def kernel(x, U, bU, V, bV):
    x = np.asarray(x, np.float32)
    U = np.asarray(U, np.float32)
    bU = np.asarray(bU, np.float32)
    V = np.asarray(V, np.float32)
    bV = np.asarray(bV, np.float32)

    # ---- host prep: contiguous-DMA layouts ----
    # Xh: [i, s128, chunk, b], Uh: [i, s128, chunk, o]
    Xh = x.reshape(B, CI, NCH, 128).transpose(1, 3, 2, 0)
    Uh = U.reshape(CO, CI, NCH, 128).transpose(1, 3, 2, 0)

    in_maps_a = []
    for c in range(N_CORES):
        sl = slice(c * I_PER_CORE, (c + 1) * I_PER_CORE)
        in_maps_a.append({
            "uh": np.ascontiguousarray(Uh[sl]),
            "xh": np.ascontiguousarray(Xh[sl]),
        })

    nc_a = _get("a")
    res_a = _run(nc_a, in_maps_a)
    # u_all[o, i, b]
    u_all = np.concatenate([res_a[c]["u_out"] for c in range(N_CORES)], axis=1)

    # ---- host mid: fold biases into extra contraction rows ----
    Sst = np.empty((CO, KB, B), np.float32)
    Sst[:, :CI, :] = u_all
    Sst[:, CI, :] = 1.0
    Sst[:, CI + 1, :] = bU[:, None]

    V3 = V.reshape(S_OUT, CO, CI)
    Vh = np.empty((CO, KB, S_OUT), np.float32)
    Vh[:, :CI, :] = V3.transpose(1, 2, 0)
    Vh[:, CI, :] = bV[None, :]
    Vh[:, CI + 1, :] = V3.sum(-1).T

    in_maps_b = []
    for c in range(N_CORES):
        sl = slice(c * O_PER_CORE, (c + 1) * O_PER_CORE)
        in_maps_b.append({
            "vh": np.ascontiguousarray(Vh[sl]),
            "us": np.ascontiguousarray(Sst[sl]),
        })

    nc_b = _get("b")
    res_b = _run(nc_b, in_maps_b)
    # z_all[o, t128, tt, b]
    z_all = np.concatenate([res_b[c]["z_out"] for c in range(N_CORES)], axis=0)

    # ---- host final: z[b, o, t] with t = tt*128 + t128 ----
    z = z_all.transpose(3, 0, 2, 1).reshape(B, CO, S_OUT)
    return np.ascontiguousarray(z.reshape(B, CO, 32, 32))
